# revision 22
# baseline (speedup 1.0000x reference)
"""DDNLoss (depth distribution network focal loss) on 8 trn2 NeuronCores.

Data-parallel over B (1 image per core, B=8). v4: original (channel-
partition) layout with the v3 bottlenecks removed:

  * Rasterize in the BIN domain: host converts per-box depths to exact
    f32 LID bin indices (monotone, so min commutes with binning), raster
    is a pure min/max of small exact-in-bf16 integers -> 32 bf16 STT ops
    on DVE, column masks broadcast by DMA (PE freed, no sqrt/cast chain,
    no ACT Sqrt table load; on-chip t now matches the reference targets
    bit-exactly).
  * Select mask via tensor_scalar is_equal (supports 4x DVE mode) +
    tensor_tensor mult (2x) instead of the modeless STT.
  * Reduce matmuls quadrant-packed: 4 outputs (S/a x 2 blocks) per
    1-bank PSUM tile at partition bases 0/32/64/96 -> drains move 4
    rows per copy (0.5 col/px instead of 2), alternating DVE/ACT,
    one batched DMA per tile (stepped-partition source AP).
  * 8 chunks, deeper pools -> PE streams near-continuously.

Host sums the 8x128 partials -> scalar loss.

Measured on trn2 (axon): 101.9us vs 146.7us for v3; rel err 1.8e-05.
Later variants (v5-v10) tried S/a stream splitting, host-pre-broadcast
masks, and DMA splitting; all measured SLOWER on hardware (110-225us) --
the tile-framework in-order queues and DMA-ring descriptor scheduling
punish cross-stream reordering, so v4's simple per-chunk structure
stands.
"""

import numpy as np
from contextlib import ExitStack

import concourse.bass as bass
import concourse.bacc as bacc_mod
import concourse.tile as tile
import concourse.mybir as mybir
from concourse.bass_utils import run_bass_kernel_spmd

try:
    import ml_dtypes
    _BF16 = ml_dtypes.bfloat16
except Exception:  # pragma: no cover
    _BF16 = None

# Problem constants (hardcoded per contract)
B, C, H, W, N = 8, 81, 96, 312, 32
HW = H * W                      # 29952
NCH = 8                         # logits chunks
CH = HW // NCH                  # 3744
QB = 468                        # matmul column block (<= 512 psum bank)
TPX = 2 * QB                    # pixels per PSUM tile (936)

ALPHA = 0.25
D_MIN, D_MAX, NUM_BINS = 0.001, 60.0, 80
BIN_SIZE = 2.0 * (D_MAX - D_MIN) / (NUM_BINS * (1 + NUM_BINS))
BIGBIN = 128.0                  # empty marker; exact in bf16, > any bin
C0 = -ALPHA / float(B * HW)     # fold -alpha and global pixel normalizer

LAST_RESULTS = None


def build_program():
    f32 = mybir.dt.float32
    bf16 = mybir.dt.bfloat16
    Alu = mybir.AluOpType
    Act = mybir.ActivationFunctionType

    nc = bacc_mod.Bacc("TRN2", target_bir_lowering=False)
    logits = nc.dram_tensor("logits", [C, HW], bf16, kind="ExternalInput")
    rowpen = nc.dram_tensor("rowpen", [H, N], f32, kind="ExternalInput")
    colbin = nc.dram_tensor("colbin", [N, W], bf16, kind="ExternalInput")
    iota81 = nc.dram_tensor("iota81", [C, 1], f32, kind="ExternalInput")
    ones81 = nc.dram_tensor("ones81", [C, 1], bf16, kind="ExternalInput")
    partial = nc.dram_tensor("partial", [128, 1], f32, kind="ExternalOutput")
    tprobe = nc.dram_tensor("tprobe", [1, HW], f32, kind="ExternalOutput")

    with ExitStack() as ctx:
        tc = ctx.enter_context(tile.TileContext(nc))
        consts = ctx.enter_context(tc.tile_pool(name="consts", bufs=1))
        rast = ctx.enter_context(tc.tile_pool(name="rast", bufs=1))
        lg = ctx.enter_context(tc.tile_pool(name="lg", bufs=3))
        tb_pool = ctx.enter_context(tc.tile_pool(name="tb", bufs=2))
        ex = ctx.enter_context(tc.tile_pool(name="ex", bufs=2))
        eqp = ctx.enter_context(tc.tile_pool(name="eq", bufs=2))
        amp = ctx.enter_context(tc.tile_pool(name="am", bufs=2))
        st_pool = ctx.enter_context(tc.tile_pool(name="st", bufs=6))
        fin = ctx.enter_context(tc.tile_pool(name="fin", bufs=1))
        psu = ctx.enter_context(tc.tile_pool(name="psu", bufs=8, space="PSUM"))
        dr = ctx.enter_context(tc.tile_pool(name="dr", bufs=1, space="DRAM"))

        # ---- constants
        c_iota81 = consts.tile([C, 1], f32)
        nc.sync.dma_start(c_iota81[:], iota81[:, :])
        c_ones81 = consts.tile([C, 1], bf16)
        nc.sync.dma_start(c_ones81[:], ones81[:, :])
        c_rowpen = consts.tile([H, N], f32)
        nc.sync.dma_start(c_rowpen[:], rowpen[:, :])

        # first two logits chunks ahead of the mask broadcasts so the
        # sync ring starts them immediately (exp0/exp1 gate the S stream)
        L_pre = []
        for j in range(2):
            Lp = lg.tile([C, CH], bf16, tag="L", name=f"Lpre{j}")
            nc.sync.dma_start(Lp[:], logits[:, j * CH:(j + 1) * CH])
            L_pre.append(Lp)

        # column-mask rows broadcast to all H partitions (stride-0 DMA),
        # spread across the sync and gpsimd queues
        c_cb = []
        for n in range(N):
            cbn = rast.tile([H, W], bf16, tag=f"cb{n}")
            q = nc.sync if (n % 2 == 0) else nc.gpsimd
            q.dma_start(cbn[:], colbin[n:n + 1, :].broadcast_to((H, W)))
            c_cb.append(cbn)

        # ---- rasterize in bin domain: T(h,w) = min_n max(rowpen, colbin)
        # two interleaved DVE chains (stt is DVE-only at the ISA level)
        dmin1 = rast.tile([H, W], bf16)
        nc.vector.memset(dmin1[:], BIGBIN)
        dmin2 = rast.tile([H, W], bf16)
        nc.vector.memset(dmin2[:], BIGBIN)
        for n in range(N):
            dst = dmin1 if (n % 2 == 0) else dmin2
            nc.vector.scalar_tensor_tensor(
                out=dst[:], in0=c_cb[n][:], scalar=c_rowpen[:, n:n + 1],
                in1=dst[:], op0=Alu.max, op1=Alu.min)
        T = rast.tile([H, W], bf16)
        nc.vector.tensor_tensor(out=T[:], in0=dmin1[:], in1=dmin2[:],
                                op=Alu.min)

        # t = min(T, 80); fg = T < 100; w = 12*fg + 1   (all exact in bf16)
        tt = rast.tile([H, W], bf16)
        nc.vector.tensor_scalar(out=tt[:], in0=T[:], scalar1=80.0,
                                scalar2=None, op0=Alu.min)
        fg = rast.tile([H, W], bf16)
        nc.vector.tensor_scalar(out=fg[:], in0=T[:], scalar1=100.0,
                                scalar2=None, op0=Alu.is_lt)
        wgt = rast.tile([H, W], bf16)
        nc.vector.tensor_scalar(out=wgt[:], in0=fg[:], scalar1=12.0,
                                scalar2=1.0, op0=Alu.mult, op1=Alu.add)
        tpf = rast.tile([H, W], f32)
        nc.vector.tensor_copy(out=tpf[:], in_=tt[:])
        nc.sync.dma_start(tprobe[0:1, :], tpf[:])

        # ---- bounce t and w to DRAM in flat pixel order
        tdram = dr.tile([1, HW], bf16)
        nc.sync.dma_start(tdram[:, :], tt[:])
        wdram = dr.tile([1, HW], bf16)
        nc.sync.dma_start(wdram[:, :], wgt[:])

        # S / a rows in DRAM (row 0 = S, row 1 = a), bf16
        sadram = dr.tile([2, HW], bf16)

        # ---- stream chunks
        drain_engines = [nc.vector, nc.scalar]
        di = 0
        srcs = [None] * NCH
        dctr = [0]

        def emit_q(j, q):
            """Per (chunk, quantity): one psum tile per 1872px with 4
            blocks at bases 0/32/64/96; drain alternates DVE/ACT."""
            src = srcs[j][q]
            base = j * CH
            for k in range(0, CH, 4 * QB):
                ps = psu.tile([128, QB], mybir.dt.float32, tag="ps", bufs=8,
                              name=f"ps{j}_{q}_{k}")
                for blk in range(4):
                    o = k + blk * QB
                    nc.tensor.matmul(ps[32 * blk:32 * blk + 1, :],
                                     c_ones81[:, 0:1], src[:, o:o + QB],
                                     start=True, stop=True,
                                     tile_position=(0, 32 * blk))
                stage = st_pool.tile([128, QB], mybir.dt.bfloat16,
                                     tag="stage", name=f"stg{dctr[0]}")
                eng = drain_engines[dctr[0] % 2]
                if eng is nc.scalar:
                    eng.copy(stage[:], ps[:])
                else:
                    eng.tensor_copy(out=stage[:], in_=ps[:])
                gb = base + k
                dst = sadram[q:q + 1, gb:gb + 4 * QB].rearrange(
                    "o (b c) -> (o b) c", b=4)
                dq = nc.gpsimd if (dctr[0] % 2 == 0) else nc.sync
                dctr[0] += 1
                dq.dma_start(dst, stage[0:97:32, 0:QB])
        for j in range(NCH):
            base = j * CH
            sl = slice(base, base + CH)
            if j < 2:
                L = L_pre[j]
            else:
                L = lg.tile([C, CH], bf16, tag="L", name=f"L{j}")
                nc.sync.dma_start(L[:], logits[:, sl])
            t_b = tb_pool.tile([C, CH], bf16, tag="tb")
            nc.sync.dma_start(t_b[:], tdram[0:1, sl].broadcast_to((C, CH)))

            E = ex.tile([C, CH], bf16, tag="E")
            nc.scalar.activation(E[:], L[:], Act.Exp)

            eq = eqp.tile([C, CH], bf16, tag="eq")
            nc.vector.tensor_scalar(out=eq[:], in0=t_b[:],
                                    scalar1=c_iota81[:, 0:1], scalar2=None,
                                    op0=Alu.is_equal)
            am = amp.tile([C, CH], bf16, tag="am")
            nc.vector.tensor_tensor(out=am[:], in0=eq[:], in1=L[:],
                                    op=Alu.mult)

            srcs[j] = (E, am)
            emit_q(j, 0)            # S matmuls for chunk j
            if j >= 1:
                emit_q(j - 1, 1)    # a matmuls lag one chunk

        emit_q(NCH - 1, 1)

        # ---- reload in (128, 234) slot layout
        NG = HW // 128  # 234
        s_slot = fin.tile([128, NG], bf16)
        nc.sync.dma_start(
            s_slot[:], sadram[0:1, :].rearrange("o (p g) -> (o p) g", p=128))
        a_slot = fin.tile([128, NG], bf16)
        nc.sync.dma_start(
            a_slot[:], sadram[1:2, :].rearrange("o (p g) -> (o p) g", p=128))
        w_slot = fin.tile([128, NG], bf16)
        nc.sync.dma_start(
            w_slot[:], wdram[0:1, :].rearrange("o (p g) -> (o p) g", p=128))

        # ---- focal epilogue on (128, 234)
        # p = exp(a)/S computed while the Exp table is still loaded, so the
        # ACT engine swaps tables only once (Exp -> Ln).
        ea = fin.tile([128, NG], f32)
        nc.scalar.activation(ea[:], a_slot[:], Act.Exp)
        rS = fin.tile([128, NG], f32)
        nc.vector.reciprocal(rS[:], s_slot[:])
        pp = fin.tile([128, NG], f32)
        nc.vector.tensor_tensor(out=pp[:], in0=ea[:], in1=rS[:], op=Alu.mult)
        lnS = fin.tile([128, NG], f32)
        nc.scalar.activation(lnS[:], s_slot[:], Act.Ln)
        logp = fin.tile([128, NG], f32)
        nc.vector.tensor_tensor(out=logp[:], in0=a_slot[:], in1=lnS[:],
                                op=Alu.subtract)
        om = fin.tile([128, NG], f32)
        nc.vector.tensor_scalar(out=om[:], in0=pp[:], scalar1=-1.0,
                                scalar2=1.0, op0=Alu.mult, op1=Alu.add)
        om2 = fin.tile([128, NG], f32)
        nc.vector.tensor_tensor(out=om2[:], in0=om[:], in1=om[:], op=Alu.mult)
        t2 = fin.tile([128, NG], f32)
        nc.vector.scalar_tensor_tensor(
            out=t2[:], in0=om2[:], scalar=C0, in1=logp[:],
            op0=Alu.mult, op1=Alu.mult)
        fs = fin.tile([128, NG], f32)
        acc = fin.tile([128, 1], f32)
        nc.vector.scalar_tensor_tensor(
            out=fs[:], in0=t2[:], scalar=0.0, in1=w_slot[:],
            op0=Alu.add, op1=Alu.mult, accum_out=acc[:])
        nc.sync.dma_start(partial[:, :], acc[:])

    nc.compile()
    return nc


_CACHE = {}


def _get_program():
    if "nc" not in _CACHE:
        _CACHE["nc"] = build_program()
    return _CACHE["nc"]


def _bin_f32(d):
    """Exact f32 replication of the reference LID binning on box depths."""
    d = np.asarray(d, dtype=np.float32)
    idx = np.float32(-0.5) + np.float32(0.5) * np.sqrt(
        np.float32(1.0) + np.float32(8.0) * (d - np.float32(D_MIN))
        / np.float32(BIN_SIZE))
    invalid = (idx < 0) | (idx > NUM_BINS) | ~np.isfinite(idx)
    return np.where(invalid, NUM_BINS, idx.astype(np.int32)).astype(np.float32)


def kernel(depth_logits, gt_boxes2d, num_gt_per_img, gt_center_depth):
    global LAST_RESULTS
    dl = np.ascontiguousarray(np.asarray(depth_logits, dtype=np.float32))
    assert dl.shape == (B, C, H, W), dl.shape
    n_gt = int(num_gt_per_img)
    assert n_gt == N, n_gt
    boxes = np.asarray(gt_boxes2d, dtype=np.float32)
    depth = np.asarray(gt_center_depth, dtype=np.float32)

    u1 = np.floor(boxes[:, 0]).astype(np.int32)
    v1 = np.floor(boxes[:, 1]).astype(np.int32)
    u2 = np.ceil(boxes[:, 2]).astype(np.int32)
    v2 = np.ceil(boxes[:, 3]).astype(np.int32)
    bins = _bin_f32(depth)                                    # (B*N,)
    rows = np.arange(H)[:, None]
    cols = np.arange(W)[None, :]
    iota81 = np.arange(C, dtype=np.float32)[:, None]
    ones81 = np.ones((C, 1), dtype=_BF16)

    logits_flat = dl.reshape(B, C, HW)
    in_maps = []
    for b in range(B):
        sl = slice(b * N, (b + 1) * N)
        bv1, bv2, bu1, bu2 = v1[sl], v2[sl], u1[sl], u2[sl]
        bb = bins[sl]
        rp = np.where((rows >= bv1[None, :]) & (rows < bv2[None, :]),
                      0.0, BIGBIN).astype(np.float32)          # (H, N)
        cb = np.where((cols >= bu1[:, None]) & (cols < bu2[:, None]),
                      bb[:, None], BIGBIN).astype(_BF16)       # (N, W)
        in_maps.append({
            "logits": np.ascontiguousarray(logits_flat[b].astype(_BF16)),
            "rowpen": np.ascontiguousarray(rp),
            "colbin": np.ascontiguousarray(cb),
            "iota81": iota81,
            "ones81": ones81,
        })

    nc = _get_program()
    res = run_bass_kernel_spmd(nc, in_maps, core_ids=list(range(B)))
    LAST_RESULTS = res
    total = np.float64(0.0)
    for r in res.results:
        total += np.asarray(r["partial"], dtype=np.float64).sum()
    return np.float32(total)


if __name__ == "__main__":
    import tempfile
    from concourse.bass_utils import compile_bass_kernel
    compile_bass_kernel(_get_program(), tempfile.mkdtemp())
    print("COMPILE OK")
